# revision 14
# baseline (speedup 1.0000x reference)
"""Trainium2 Bass kernel for nn_CausalCosmosBlock (sink+sliding-window attention block).

Reference computation (B=2, L=256, D=4096, H=32, Dh=128, SINK=128, S=4224):
    q = rmsnorm((x @ Wq).reshape(B,L,H,Dh)) ; k likewise ; v = x @ Wv
    new_k = concat(cache_k[:, :128], cache_k[:, 384:], k)   # S rows
    logits = q @ new_k^T / sqrt(Dh), causal mask j <= (S-L)+i
    out = softmax(logits) @ new_v ; return out @ Wo

Sharding: tensor-parallel over heads. 8 cores x 4 heads. Each core computes
its heads' projections from the full x, attention over its heads' cache, and
a partial y = out_heads @ Wo[head_rows]; host sums the 8 partials.

Device layout choices (everything chosen so no on-device transposes needed):
  - x is fed transposed (xT [D, B*L]); projections produce qT/kT [Dh, tok].
  - K-cache fed pre-transposed per head ([Dh, S_keep]); logits computed as
    logits^T [s, l] chunks so softmax sums reduce over partitions via a
    ones-vector matmul, and attn@V uses V in natural [s, d] layout as lhsT.
  - V-cache fed as [S_keep, 4*Dh] (head-interleaved); new-chunk V computed
    token-major directly.
All matmul operands fp16 (1 cycle/row on PE); accumulation fp32 in PSUM;
softmax stats and final output fp32.

DMAs are batched (4 row-blocks per transfer) and split across both HWDGE
rings (SP ring: weights/x/output; ACT ring: kv-cache) — per-DMA HWDGE ring
occupancy is the main non-PE bottleneck.
"""

import contextlib

import numpy as np

import concourse.bass as bass
import concourse.tile as tile
import concourse.mybir as mybir
from concourse import bacc
from concourse.bass_utils import run_bass_kernel_spmd

# Problem shapes (hardcoded per contract)
B = 2
L = 256
D = 4096
H = 32
Dh = 128
SINK = 128
S = 4224
KEEP = SINK + (S - SINK - L)  # 3968 old cache rows kept
ST = KEEP // 128  # 31 old s-tiles of 128
KT = D // 128  # 32 contraction tiles
TOK = B * L  # 512
N_CORES = 8
HPC = H // N_CORES  # 4 heads per core
EPS = 1e-6
SCALE = 1.0 / float(np.sqrt(Dh))

DT = mybir.dt.float16
DT_NP = np.float16
F32 = mybir.dt.float32
F32R = mybir.dt.float32r

GRP = 4  # row-blocks per batched DMA


def _declare_io(nc, with_reps=False):
    t = {}
    t["xT"] = nc.dram_tensor("xT", [D, TOK], DT, kind="ExternalInput")
    t["wq"] = nc.dram_tensor("wq", [D, HPC * Dh], DT, kind="ExternalInput")
    t["wk"] = nc.dram_tensor("wk", [D, HPC * Dh], DT, kind="ExternalInput")
    t["wv"] = nc.dram_tensor("wv", [D, HPC * Dh], DT, kind="ExternalInput")
    # wo[p, h, d] = Wo[head_base + h*128 + p, d]
    t["wo"] = nc.dram_tensor("wo", [128, HPC, D], DT, kind="ExternalInput")
    t["kTc"] = nc.dram_tensor("kTc", [B, HPC, Dh, KEEP], DT, kind="ExternalInput")
    t["vc"] = nc.dram_tensor("vc", [B, KEEP, HPC * Dh], DT, kind="ExternalInput")
    t["qw"] = nc.dram_tensor("qw", [1, Dh], F32, kind="ExternalInput")
    t["kw"] = nc.dram_tensor("kw", [1, Dh], F32, kind="ExternalInput")
    t["maskt"] = nc.dram_tensor("maskt", [128, 2, L], DT, kind="ExternalInput")
    if with_reps:
        t["reps"] = nc.dram_tensor("reps", [1, 1], mybir.dt.uint32, kind="ExternalInput")
    t["yT"] = nc.dram_tensor("yT", [D, TOK], F32, kind="ExternalOutput")
    return t


def _grouped_rows(ap2d, grp=GRP):
    """View a [n*grp*128, width] DRAM AP as [g][128(p), j, width] row groups."""
    return ap2d.rearrange("(g j p) f -> g p j f", p=128, j=grp)


def _emit_body(nc, tc, t, consts, pools):
    """Emit one full forward pass. `consts` holds preloaded tiny const tiles."""
    qw_sb, kw_sb, mask_sb, ones_col16, ones_col32, ones_row32, eps_sb = consts

    # ---- batched resident loads (SP ring) ----
    xT_g = _grouped_rows(t["xT"])  # [8][128, 4, TOK]
    xt_groups = []
    for g in range(KT // GRP):
        xg = pools["xt"].tile([128, GRP, TOK], DT, tag=f"xt{g}", name=f"xt{g}")
        nc.sync.dma_start(xg[:], xT_g[g])
        xt_groups.append(xg)

    def xt_tile(kt):
        return xt_groups[kt // GRP][:, kt % GRP, :]

    wo_sb = pools["wo"].tile([128, HPC, D], DT, tag="wo", name="wo_sb")
    nc.sync.dma_start(wo_sb[:], t["wo"][:])

    # ---------------- Phase B: projections ----------------
    qn_sb = [None] * HPC
    kn_sb = [None] * HPC
    with (
        tc.tile_pool(name="pp_psum", bufs=5, space="PSUM") as pp_psum,
        tc.tile_pool(name="ss_psum", bufs=1, space="PSUM") as ss_psum,
        tc.tile_pool(name="winv_psum", bufs=2, space="PSUM") as winv_psum,
        tc.tile_pool(name="wslab", bufs=3) as wslab_pool,
        tc.tile_pool(name="praw", bufs=3) as praw_pool,
        tc.tile_pool(name="pstat", bufs=4) as pstat_pool,
    ):
        qkv_sb = pools["qkv"]
        vnew_sb = qkv_sb.tile([128, 2 * B, HPC * Dh], DT, tag="vnew", name="vnew")

        for proj_i, (wt, normw) in enumerate(((t["wq"], qw_sb), (t["wk"], kw_sb))):
            wt_g = _grouped_rows(wt)
            ps = [pp_psum.tile([128, TOK], F32, tag="pp", name=f"pp{_h}") for _h in range(HPC)]
            for g in range(KT // GRP):
                slab = wslab_pool.tile([128, GRP, HPC * Dh], DT, tag="w", name=f"w{proj_i}_{g}")
                nc.sync.dma_start(slab[:], wt_g[g])
                for j in range(GRP):
                    kt = g * GRP + j
                    for h in range(HPC):
                        nc.tensor.matmul(
                            ps[h][:],
                            slab[:, j, h * Dh : (h + 1) * Dh],
                            xt_tile(kt),
                            start=(kt == 0),
                            stop=(kt == KT - 1),
                        )
            for h in range(HPC):
                # rmsnorm over Dh (= partitions) via ones-matmul reduction
                raw = praw_pool.tile([128, TOK], F32, tag="raw", name=f"raw{proj_i}{h}")
                nc.scalar.copy(raw[:], ps[h][:])
                sq = praw_pool.tile([128, TOK], F32R, tag="sq", name=f"sq{proj_i}{h}")
                nc.vector.tensor_mul(sq[:], raw[:], raw[:])
                ss_ps = ss_psum.tile([1, TOK], F32, tag="ss", name=f"ss{proj_i}{h}")
                nc.tensor.matmul(ss_ps[:], ones_col32[:], sq[:], start=True, stop=True)
                # inv = 1/sqrt(ss/Dh + eps)
                rms = pstat_pool.tile([1, TOK], F32, tag="rms", name=f"rms{proj_i}{h}")
                nc.scalar.activation(
                    rms[:],
                    ss_ps[:],
                    mybir.ActivationFunctionType.Sqrt,
                    scale=1.0 / Dh,
                    bias=eps_sb[0:1, 0:1],
                )
                inv = pstat_pool.tile([1, TOK], F32R, tag="inv", name=f"inv{proj_i}{h}")
                nc.vector.reciprocal(inv[:], rms[:])
                # winv[d, t] = norm_w[d] * inv[t]  (rank-1 outer product on PE)
                winv_ps = winv_psum.tile([128, TOK], F32, tag="winv", name=f"wi{proj_i}{h}")
                nc.tensor.matmul(winv_ps[:], normw[:], inv[:], start=True, stop=True)
                normed = qkv_sb.tile([128, TOK], DT, tag=f"qk{proj_i}{h}", name=f"qk{proj_i}{h}")
                nc.vector.tensor_mul(normed[:], raw[:], winv_ps[:])
                if proj_i == 0:
                    qn_sb[h] = normed
                else:
                    kn_sb[h] = normed

        # V projection, token-major: out[t, f] for 2B token chunks
        wv_g = _grouped_rows(t["wv"])
        vps = [pp_psum.tile([128, HPC * Dh], F32, tag="pp", name=f"vp{_c}") for _c in range(2 * B)]
        for g in range(KT // GRP):
            slab = wslab_pool.tile([128, GRP, HPC * Dh], DT, tag="w", name=f"wv_{g}")
            nc.sync.dma_start(slab[:], wv_g[g])
            for j in range(GRP):
                kt = g * GRP + j
                for c in range(2 * B):
                    nc.tensor.matmul(
                        vps[c][:],
                        xt_tile(kt)[:, c * 128 : (c + 1) * 128],
                        slab[:, j, :],
                        start=(kt == 0),
                        stop=(kt == KT - 1),
                    )
        for c in range(2 * B):
            nc.scalar.copy(vnew_sb[:, c, :], vps[c][:])

    # ---------------- Phase C: attention + Phase D: output projection ----------------
    GRPA = 2  # attention chunks per exp group (one PSUM bank)
    groups = []
    ci = 0
    while ci < ST:
        n = min(GRPA, ST - ci)
        groups.append(("old", tuple(range(ci, ci + n))))
        ci += n
    groups.append(("new", (0, 1)))

    VG = (ST + GRP - 1) // GRP  # v-cache DMA groups per batch

    with (
        tc.tile_pool(name="lg_psum", bufs=3, space="PSUM") as lg_psum,
        tc.tile_pool(name="oacc_psum", bufs=2, space="PSUM") as oacc_psum,
        tc.tile_pool(name="sum_psum", bufs=1, space="PSUM") as sum_psum,
        tc.tile_pool(name="wo_psum", bufs=2, space="PSUM") as wo_psum,
        tc.tile_pool(name="vslab", bufs=VG + 1) as vslab_pool,
        tc.tile_pool(name="ktslab", bufs=2) as kt_pool,
        tc.tile_pool(name="pexp", bufs=3) as pexp_pool,
        tc.tile_pool(name="attn_sm", bufs=4) as attn_sm,
        tc.tile_pool(name="outT", bufs=2) as outT_pool,
        tc.tile_pool(name="ysb", bufs=3) as ysb_pool,
    ):
        for b in range(B):
            # batched v-cache loads on the ACT HWDGE ring
            v_groups = []
            for g in range(VG):
                j0 = g * GRP
                jn = min(GRP, ST - j0)
                vg = vslab_pool.tile([128, GRP, HPC * Dh], DT, tag="v", name=f"v{b}_{g}")
                src = t["vc"][b, j0 * 128 : (j0 + jn) * 128, :].rearrange(
                    "(j p) f -> p j f", p=128
                )
                nc.scalar.dma_start(vg[:, :jn, :], src)
                v_groups.append(vg)

            def v_lhsT_old(cidx, h):
                return v_groups[cidx // GRP][:, cidx % GRP, h * Dh : (h + 1) * Dh]

            outT_b = outT_pool.tile([128, HPC, L], DT, tag="outT", name=f"outT{b}")

            for h in range(HPC):
                kt_slab = kt_pool.tile([Dh, KEEP], DT, tag="kt", name=f"kt{b}{h}")
                nc.scalar.dma_start(kt_slab[:], t["kTc"][b, h])

                oacc = oacc_psum.tile([128, L], F32, tag="oacc", name=f"oacc{b}{h}")
                out_ps = oacc[:]
                sum_t = sum_psum.tile([1, L], F32, tag="sum", name=f"sum{b}{h}")
                sum_ps = sum_t[:]
                q_rhs = qn_sb[h][:, b * L : (b + 1) * L]

                n_chunks = ST + 2
                chunk_idx = 0
                for gi, (kind, chunks) in enumerate(groups):
                    w = len(chunks)
                    lg = lg_psum.tile([128, GRPA, L], F32, tag="lg", name=f"lg{b}{h}{gi}")
                    for j, cidx in enumerate(chunks):
                        if kind == "old":
                            lhsT = kt_slab[:, cidx * 128 : (cidx + 1) * 128]
                        else:
                            lhsT = kn_sb[h][:, b * L + cidx * 128 : b * L + (cidx + 1) * 128]
                        nc.tensor.matmul(lg[:, j, :], lhsT, q_rhs, start=True, stop=True)
                    pexp = pexp_pool.tile([128, GRPA, L], DT, tag="pexp", name=f"pe{b}{h}{gi}")
                    nc.scalar.activation(
                        pexp[:, :w, :],
                        lg[:, :w, :],
                        mybir.ActivationFunctionType.Exp,
                        scale=SCALE,
                    )
                    if kind == "new":
                        nc.vector.tensor_mul(pexp[:, :2, :], pexp[:, :2, :], mask_sb[:])
                    for j, cidx in enumerate(chunks):
                        first = chunk_idx == 0
                        last = chunk_idx == n_chunks - 1
                        nc.tensor.matmul(
                            sum_ps, ones_col16[:], pexp[:, j, :], start=first, stop=last
                        )
                        if kind == "old":
                            v_lhsT = v_lhsT_old(cidx, h)
                        else:
                            v_lhsT = vnew_sb[:, b * 2 + cidx, h * Dh : (h + 1) * Dh]
                        nc.tensor.matmul(
                            out_ps, v_lhsT, pexp[:, j, :], start=first, stop=last
                        )
                        chunk_idx += 1

                # normalize: outT[:, h, l] = out_ps[:, l] / sum_ps[l]
                recip = attn_sm.tile([1, L], F32R, tag="recip", name=f"rc{b}{h}")
                nc.vector.reciprocal(recip[:], sum_ps)
                bc_t = lg_psum.tile([128, GRPA, L], F32, tag="lg", name=f"bc{b}{h}")
                bc_ps = bc_t[:, 0, :]
                nc.tensor.matmul(bc_ps, ones_row32[:], recip[:], start=True, stop=True)
                ocopy = attn_sm.tile([128, L], F32, tag="ocopy", name=f"oc{b}{h}")
                nc.vector.tensor_copy(ocopy[:], out_ps)
                nc.vector.tensor_mul(outT_b[:, h, :], ocopy[:], bc_ps)

            # Output projection for this batch (staged into 4-block groups)
            yT_g = _grouped_rows(t["yT"])
            for g in range(KT // GRP):
                y_sb = ysb_pool.tile([128, GRP, L], F32, tag="ysb", name=f"y{b}_{g}")
                for j in range(GRP):
                    m = g * GRP + j
                    y_ps = wo_psum.tile([128, L], F32, tag="y", name=f"yp{b}_{m}")
                    for h in range(HPC):
                        nc.tensor.matmul(
                            y_ps[:],
                            wo_sb[:, h, m * 128 : (m + 1) * 128],
                            outT_b[:, h, :],
                            start=(h == 0),
                            stop=(h == HPC - 1),
                        )
                    nc.vector.tensor_copy(y_sb[:, j, :], y_ps[:])
                nc.sync.dma_start(yT_g[g][:, :, b * L : (b + 1) * L], y_sb[:])


def build_program(reps_loop=False):
    nc = bacc.Bacc(
        "TRN2",
        target_bir_lowering=False,
        debug=False,
        enable_asserts=False,
        num_devices=N_CORES,
    )
    t = _declare_io(nc, with_reps=reps_loop)

    with (
        nc.allow_low_precision(reason="deliberate fp16/fp32r compute"),
        tile.TileContext(nc) as tc,
    ):
        with contextlib.ExitStack() as ctx:
            consts_pool = ctx.enter_context(tc.tile_pool(name="consts", bufs=1))
            xt_pool = ctx.enter_context(tc.tile_pool(name="xt", bufs=1))
            wo_pool = ctx.enter_context(tc.tile_pool(name="wop", bufs=1))
            qkv_pool = ctx.enter_context(tc.tile_pool(name="qkv", bufs=2))

            qw_ld = consts_pool.tile([1, Dh], F32)
            nc.sync.dma_start(qw_ld[:], t["qw"][:])
            qw_sb = consts_pool.tile([1, Dh], F32R)
            nc.vector.tensor_copy(qw_sb[:], qw_ld[:])
            kw_ld = consts_pool.tile([1, Dh], F32)
            nc.sync.dma_start(kw_ld[:], t["kw"][:])
            kw_sb = consts_pool.tile([1, Dh], F32R)
            nc.vector.tensor_copy(kw_sb[:], kw_ld[:])
            mask_sb = consts_pool.tile([128, 2, L], DT)
            nc.sync.dma_start(mask_sb[:], t["maskt"][:])
            ones_col16 = consts_pool.tile([128, 1], DT)
            nc.vector.memset(ones_col16[:], 1.0)
            ones_stage = consts_pool.tile([128, 128], F32)
            nc.vector.memset(ones_stage[:], 1.0)
            ones_col32 = consts_pool.tile([128, 1], F32R)
            nc.vector.tensor_copy(ones_col32[:], ones_stage[:, 0:1])
            ones_row32 = consts_pool.tile([1, 128], F32R)
            nc.vector.tensor_copy(ones_row32[:], ones_stage[0:1, :])
            eps_sb = consts_pool.tile([1, 1], F32)
            nc.vector.memset(eps_sb[:], EPS)

            consts = (qw_sb, kw_sb, mask_sb, ones_col16, ones_col32, ones_row32, eps_sb)
            pools = {"qkv": qkv_pool, "xt": xt_pool, "wo": wo_pool}

            if reps_loop:
                reps_sb = consts_pool.tile([1, 1], mybir.dt.uint32)
                nc.sync.dma_start(reps_sb[:], t["reps"][:])
                reps_regs = nc.alloc_registers("reps_regs")
                nc.regs_load(reps_regs, reps_sb[0:1, 0:1])
                reps_val = nc.snap(reps_regs, donate=True, min_val=1, max_val=1 << 20)
                with tc.For_i(0, reps_val, 1):
                    _emit_body(nc, tc, t, consts, pools)
            else:
                _emit_body(nc, tc, t, consts, pools)

    nc.compile()
    return nc


def prep_inputs(x, cache_k, cache_v, Wq, Wk, Wv, Wo, q_norm_w, k_norm_w, sink):
    """Host-side sharding/layout prep. Returns in_maps for the 8 cores."""
    x = np.asarray(x, dtype=np.float32)
    cache_k = np.asarray(cache_k, dtype=np.float32)
    cache_v = np.asarray(cache_v, dtype=np.float32)
    Wq = np.asarray(Wq, dtype=np.float32)
    Wk = np.asarray(Wk, dtype=np.float32)
    Wv = np.asarray(Wv, dtype=np.float32)
    Wo = np.asarray(Wo, dtype=np.float32)
    q_norm_w = np.asarray(q_norm_w, dtype=np.float32)
    k_norm_w = np.asarray(k_norm_w, dtype=np.float32)
    sink = int(sink)
    assert sink == SINK, f"kernel hardcodes sink={SINK}, got {sink}"

    xT = np.ascontiguousarray(x.reshape(TOK, D).T.astype(DT_NP))  # [D, TOK]

    # kept old-cache rows: [0:sink] ++ [sink+L : S]
    ck = np.concatenate([cache_k[:, :SINK], cache_k[:, SINK + L :]], axis=1)
    cv = np.concatenate([cache_v[:, :SINK], cache_v[:, SINK + L :]], axis=1)

    qw = np.ascontiguousarray(q_norm_w.reshape(1, Dh))
    kw = np.ascontiguousarray(k_norm_w.reshape(1, Dh))

    # mask[p, c, l] = 1 if new-chunk position c*128+p is visible to query l
    pi = np.arange(128)[:, None, None]
    cc = np.arange(2)[None, :, None]
    ll = np.arange(L)[None, None, :]
    maskt = ((cc * 128 + pi) <= ll).astype(DT_NP)

    in_maps = []
    for core in range(N_CORES):
        hs = core * HPC
        fs = hs * Dh  # feature start column
        fe = fs + HPC * Dh
        wq_c = np.ascontiguousarray(Wq[:, fs:fe].astype(DT_NP))
        wk_c = np.ascontiguousarray(Wk[:, fs:fe].astype(DT_NP))
        wv_c = np.ascontiguousarray(Wv[:, fs:fe].astype(DT_NP))
        wo_c = np.ascontiguousarray(
            Wo[fs:fe, :].reshape(HPC, 128, D).transpose(1, 0, 2).astype(DT_NP)
        )  # [128, HPC, D]
        kTc = np.ascontiguousarray(
            ck[:, :, hs : hs + HPC, :].transpose(0, 2, 3, 1).astype(DT_NP)
        )  # [B, HPC, Dh, KEEP]
        vc = np.ascontiguousarray(
            cv[:, :, hs : hs + HPC, :].reshape(B, KEEP, HPC * Dh).astype(DT_NP)
        )  # [B, KEEP, HPC*Dh]
        in_maps.append(
            {
                "xT": xT,
                "wq": wq_c,
                "wk": wk_c,
                "wv": wv_c,
                "wo": wo_c,
                "kTc": kTc,
                "vc": vc,
                "qw": qw,
                "kw": kw,
                "maskt": maskt,
            }
        )
    return in_maps


_PROGRAM_CACHE = {}


def _get_program(reps_loop=False):
    key = bool(reps_loop)
    if key not in _PROGRAM_CACHE:
        _PROGRAM_CACHE[key] = build_program(reps_loop=key)
    return _PROGRAM_CACHE[key]


def kernel(**inputs) -> np.ndarray:
    in_maps = prep_inputs(**inputs)
    nc = _get_program(reps_loop=False)
    res = run_bass_kernel_spmd(nc, in_maps, core_ids=list(range(N_CORES)))
    yT = np.zeros((D, TOK), np.float64)
    for c in range(N_CORES):
        yT += res.results[c]["yT"].astype(np.float64)
    y = yT.T.reshape(B, L, D).astype(np.float32)
    return y


if __name__ == "__main__":
    rng = np.random.default_rng(0)
    inputs = {
        "x": rng.standard_normal((B, L, D), dtype=np.float32),
        "cache_k": rng.standard_normal((B, S, H, Dh), dtype=np.float32),
        "cache_v": rng.standard_normal((B, S, H, Dh), dtype=np.float32),
        "Wq": (rng.standard_normal((D, D), dtype=np.float32) * 0.02),
        "Wk": (rng.standard_normal((D, D), dtype=np.float32) * 0.02),
        "Wv": (rng.standard_normal((D, D), dtype=np.float32) * 0.02),
        "Wo": (rng.standard_normal((D, D), dtype=np.float32) * 0.02),
        "q_norm_w": np.ones(Dh, np.float32),
        "k_norm_w": np.ones(Dh, np.float32),
        "sink": SINK,
    }
    y = kernel(**inputs)
    print("y", y.shape, y.dtype, float(np.abs(y).mean()))


# revision 15
# speedup vs baseline: 1.0556x; 1.0556x over previous
"""Trainium2 Bass kernel for nn_CausalCosmosBlock (sink+sliding-window attention block).

Reference computation (B=2, L=256, D=4096, H=32, Dh=128, SINK=128, S=4224):
    q = rmsnorm((x @ Wq).reshape(B,L,H,Dh)) ; k likewise ; v = x @ Wv
    new_k = concat(cache_k[:, :128], cache_k[:, 384:], k)   # S rows
    logits = q @ new_k^T / sqrt(Dh), causal mask j <= (S-L)+i
    out = softmax(logits) @ new_v ; return out @ Wo

Sharding: tensor-parallel over heads. 8 cores x 4 heads. Each core computes
its heads' projections from the full x, attention over its heads' cache, and
a partial y = out_heads @ Wo[head_rows]; host sums the 8 partials.

Device layout choices (everything chosen so no on-device transposes needed):
  - x is fed transposed (xT [D, B*L]); projections produce qT/kT [Dh, tok].
  - K-cache fed pre-transposed per head ([Dh, S_keep]); logits computed as
    logits^T [s, l] chunks so softmax sums reduce over partitions via a
    ones-vector matmul, and attn@V uses V in natural [s, d] layout as lhsT.
  - V-cache fed as [S_keep, 4*Dh] (head-interleaved); new-chunk V computed
    token-major directly.
All matmul operands fp16 (1 cycle/row on PE); accumulation fp32 in PSUM;
softmax stats and final output fp32.

DMAs are batched (4 row-blocks per transfer) and split across both HWDGE
rings (SP ring: weights/x/output; ACT ring: kv-cache) — per-DMA HWDGE ring
occupancy is the main non-PE bottleneck.
"""

import contextlib

import numpy as np

import concourse.bass as bass
import concourse.tile as tile
import concourse.mybir as mybir
from concourse import bacc
from concourse.bass_utils import run_bass_kernel_spmd

# Problem shapes (hardcoded per contract)
B = 2
L = 256
D = 4096
H = 32
Dh = 128
SINK = 128
S = 4224
KEEP = SINK + (S - SINK - L)  # 3968 old cache rows kept
ST = KEEP // 128  # 31 old s-tiles of 128
KT = D // 128  # 32 contraction tiles
TOK = B * L  # 512
N_CORES = 8
HPC = H // N_CORES  # 4 heads per core
EPS = 1e-6
SCALE = 1.0 / float(np.sqrt(Dh))

DT = mybir.dt.float16
DT_NP = np.float16
F32 = mybir.dt.float32
F32R = mybir.dt.float32r

GRP = 4  # row-blocks per batched DMA


def _declare_io(nc, with_reps=False):
    t = {}
    t["xT"] = nc.dram_tensor("xT", [D, TOK], DT, kind="ExternalInput")
    t["wq"] = nc.dram_tensor("wq", [D, HPC * Dh], DT, kind="ExternalInput")
    t["wk"] = nc.dram_tensor("wk", [D, HPC * Dh], DT, kind="ExternalInput")
    t["wv"] = nc.dram_tensor("wv", [D, HPC * Dh], DT, kind="ExternalInput")
    # wo[p, h, d] = Wo[head_base + h*128 + p, d]
    t["wo"] = nc.dram_tensor("wo", [128, HPC, D], DT, kind="ExternalInput")
    t["kTc"] = nc.dram_tensor("kTc", [B, HPC, Dh, KEEP], DT, kind="ExternalInput")
    t["vc"] = nc.dram_tensor("vc", [B, KEEP, HPC * Dh], DT, kind="ExternalInput")
    t["qw"] = nc.dram_tensor("qw", [1, Dh], F32, kind="ExternalInput")
    t["kw"] = nc.dram_tensor("kw", [1, Dh], F32, kind="ExternalInput")
    t["maskt"] = nc.dram_tensor("maskt", [128, 2, L], DT, kind="ExternalInput")
    if with_reps:
        t["reps"] = nc.dram_tensor("reps", [1, 1], mybir.dt.uint32, kind="ExternalInput")
    t["yT"] = nc.dram_tensor("yT", [D, TOK], F32, kind="ExternalOutput")
    return t


def _grouped_rows(ap2d, grp=GRP):
    """View a [n*grp*128, width] DRAM AP as [g][128(p), j, width] row groups."""
    return ap2d.rearrange("(g j p) f -> g p j f", p=128, j=grp)


def _emit_body(nc, tc, t, consts, pools):
    """Emit one full forward pass. `consts` holds preloaded tiny const tiles."""
    qw_sb, kw_sb, mask_sb, ones_col16, ones_col32, ones_row32, eps_sb = consts

    # ---- batched resident loads (SP ring) ----
    xT_g = _grouped_rows(t["xT"])  # [8][128, 4, TOK]
    xt_groups = []
    for g in range(KT // GRP):
        xg = pools["xt"].tile([128, GRP, TOK], DT, tag=f"xt{g}", name=f"xt{g}")
        nc.sync.dma_start(xg[:], xT_g[g])
        xt_groups.append(xg)

    def xt_tile(kt):
        return xt_groups[kt // GRP][:, kt % GRP, :]

    wo_sb = pools["wo"].tile([128, HPC, D], DT, tag="wo", name="wo_sb")
    nc.sync.dma_start(wo_sb[:], t["wo"][:])

    # ---------------- Phase B: projections ----------------
    qn_sb = [None] * HPC
    kn_sb = [None] * HPC
    with (
        tc.tile_pool(name="pp_psum", bufs=5, space="PSUM") as pp_psum,
        tc.tile_pool(name="ss_psum", bufs=1, space="PSUM") as ss_psum,
        tc.tile_pool(name="winv_psum", bufs=2, space="PSUM") as winv_psum,
        tc.tile_pool(name="wslab", bufs=3) as wslab_pool,
        tc.tile_pool(name="praw", bufs=3) as praw_pool,
        tc.tile_pool(name="pstat", bufs=4) as pstat_pool,
    ):
        qkv_sb = pools["qkv"]
        vnew_sb = qkv_sb.tile([128, 2 * B, HPC * Dh], DT, tag="vnew", name="vnew")

        for proj_i, (wt, normw) in enumerate(((t["wq"], qw_sb), (t["wk"], kw_sb))):
            wt_g = _grouped_rows(wt)
            ps = [pp_psum.tile([128, TOK], F32, tag="pp", name=f"pp{_h}") for _h in range(HPC)]
            for g in range(KT // GRP):
                slab = wslab_pool.tile([128, GRP, HPC * Dh], DT, tag="w", name=f"w{proj_i}_{g}")
                nc.sync.dma_start(slab[:], wt_g[g])
                for j in range(GRP):
                    kt = g * GRP + j
                    for h in range(HPC):
                        nc.tensor.matmul(
                            ps[h][:],
                            slab[:, j, h * Dh : (h + 1) * Dh],
                            xt_tile(kt),
                            start=(kt == 0),
                            stop=(kt == KT - 1),
                        )
            for h in range(HPC):
                # rmsnorm over Dh (= partitions) via ones-matmul reduction
                raw = praw_pool.tile([128, TOK], F32, tag="raw", name=f"raw{proj_i}{h}")
                nc.scalar.copy(raw[:], ps[h][:])
                sq = praw_pool.tile([128, TOK], F32R, tag="sq", name=f"sq{proj_i}{h}")
                nc.vector.tensor_mul(sq[:], raw[:], raw[:])
                ss_ps = ss_psum.tile([1, TOK], F32, tag="ss", name=f"ss{proj_i}{h}")
                nc.tensor.matmul(ss_ps[:], ones_col32[:], sq[:], start=True, stop=True)
                # inv = 1/sqrt(ss/Dh + eps)
                rms = pstat_pool.tile([1, TOK], F32, tag="rms", name=f"rms{proj_i}{h}")
                nc.scalar.activation(
                    rms[:],
                    ss_ps[:],
                    mybir.ActivationFunctionType.Sqrt,
                    scale=1.0 / Dh,
                    bias=eps_sb[0:1, 0:1],
                )
                inv = pstat_pool.tile([1, TOK], F32R, tag="inv", name=f"inv{proj_i}{h}")
                nc.vector.reciprocal(inv[:], rms[:])
                # winv[d, t] = norm_w[d] * inv[t]  (rank-1 outer product on PE)
                winv_ps = winv_psum.tile([128, TOK], F32, tag="winv", name=f"wi{proj_i}{h}")
                nc.tensor.matmul(winv_ps[:], normw[:], inv[:], start=True, stop=True)
                normed = qkv_sb.tile([128, TOK], DT, tag=f"qk{proj_i}{h}", name=f"qk{proj_i}{h}")
                nc.vector.tensor_mul(normed[:], raw[:], winv_ps[:])
                if proj_i == 0:
                    qn_sb[h] = normed
                else:
                    kn_sb[h] = normed

        # V projection, token-major: out[t, f] for 2B token chunks
        wv_g = _grouped_rows(t["wv"])
        vps = [pp_psum.tile([128, HPC * Dh], F32, tag="pp", name=f"vp{_c}") for _c in range(2 * B)]
        for g in range(KT // GRP):
            slab = wslab_pool.tile([128, GRP, HPC * Dh], DT, tag="w", name=f"wv_{g}")
            nc.sync.dma_start(slab[:], wv_g[g])
            for j in range(GRP):
                kt = g * GRP + j
                for c in range(2 * B):
                    nc.tensor.matmul(
                        vps[c][:],
                        xt_tile(kt)[:, c * 128 : (c + 1) * 128],
                        slab[:, j, :],
                        start=(kt == 0),
                        stop=(kt == KT - 1),
                    )
        for c in range(2 * B):
            nc.scalar.copy(vnew_sb[:, c, :], vps[c][:])

    # ---------------- Phase C: attention + Phase D: output projection ----------------
    GRPA = 2  # attention chunks per exp group (one PSUM bank)
    groups = []
    ci = 0
    while ci < ST:
        n = min(GRPA, ST - ci)
        groups.append(("old", tuple(range(ci, ci + n))))
        ci += n
    groups.append(("new", (0, 1)))

    VG = (ST + GRP - 1) // GRP  # v-cache DMA groups per batch

    with (
        tc.tile_pool(name="lg_psum", bufs=2, space="PSUM") as lg_psum,
        tc.tile_pool(name="oacc_psum", bufs=2, space="PSUM") as oacc_psum,
        tc.tile_pool(name="sum_psum", bufs=1, space="PSUM") as sum_psum,
        tc.tile_pool(name="bc_psum", bufs=1, space="PSUM") as bc_psum,
        tc.tile_pool(name="wo_psum", bufs=2, space="PSUM") as wo_psum,
        tc.tile_pool(name="vslab", bufs=VG + 1) as vslab_pool,
        tc.tile_pool(name="ktslab", bufs=2) as kt_pool,
        tc.tile_pool(name="pexp", bufs=3) as pexp_pool,
        tc.tile_pool(name="attn_sm", bufs=4) as attn_sm,
        tc.tile_pool(name="outT", bufs=2) as outT_pool,
        tc.tile_pool(name="ysb", bufs=3) as ysb_pool,
    ):
        for b in range(B):
            # batched v-cache loads on the ACT HWDGE ring
            v_groups = []
            for g in range(VG):
                j0 = g * GRP
                jn = min(GRP, ST - j0)
                vg = vslab_pool.tile([128, GRP, HPC * Dh], DT, tag="v", name=f"v{b}_{g}")
                src = t["vc"][b, j0 * 128 : (j0 + jn) * 128, :].rearrange(
                    "(j p) f -> p j f", p=128
                )
                nc.gpsimd.dma_start(vg[:, :jn, :], src)
                v_groups.append(vg)

            def v_lhsT_old(cidx, h):
                return v_groups[cidx // GRP][:, cidx % GRP, h * Dh : (h + 1) * Dh]

            outT_b = outT_pool.tile([128, HPC, L], DT, tag="outT", name=f"outT{b}")

            for h in range(HPC):
                kt_slab = kt_pool.tile([Dh, KEEP], DT, tag="kt", name=f"kt{b}{h}")
                nc.gpsimd.dma_start(kt_slab[:], t["kTc"][b, h])

                oacc = oacc_psum.tile([128, L], F32, tag="oacc", name=f"oacc{b}{h}")
                out_ps = oacc[:]
                sum_t = sum_psum.tile([1, L], F32, tag="sum", name=f"sum{b}{h}")
                sum_ps = sum_t[:]
                q_rhs = qn_sb[h][:, b * L : (b + 1) * L]

                n_chunks = ST + 2
                chunk_idx = 0
                for gi, (kind, chunks) in enumerate(groups):
                    w = len(chunks)
                    lg = lg_psum.tile([128, GRPA, L], F32, tag="lg", name=f"lg{b}{h}{gi}")
                    for j, cidx in enumerate(chunks):
                        if kind == "old":
                            lhsT = kt_slab[:, cidx * 128 : (cidx + 1) * 128]
                        else:
                            lhsT = kn_sb[h][:, b * L + cidx * 128 : b * L + (cidx + 1) * 128]
                        nc.tensor.matmul(lg[:, j, :], lhsT, q_rhs, start=True, stop=True)
                    pexp = pexp_pool.tile([128, GRPA, L], DT, tag="pexp", name=f"pe{b}{h}{gi}")
                    nc.scalar.activation(
                        pexp[:, :w, :],
                        lg[:, :w, :],
                        mybir.ActivationFunctionType.Exp,
                        scale=SCALE,
                    )
                    if kind == "new":
                        nc.vector.tensor_mul(pexp[:, :2, :], pexp[:, :2, :], mask_sb[:])
                    for j, cidx in enumerate(chunks):
                        first = chunk_idx == 0
                        last = chunk_idx == n_chunks - 1
                        nc.tensor.matmul(
                            sum_ps, ones_col16[:], pexp[:, j, :], start=first, stop=last
                        )
                        if kind == "old":
                            v_lhsT = v_lhsT_old(cidx, h)
                        else:
                            v_lhsT = vnew_sb[:, b * 2 + cidx, h * Dh : (h + 1) * Dh]
                        nc.tensor.matmul(
                            out_ps, v_lhsT, pexp[:, j, :], start=first, stop=last
                        )
                        chunk_idx += 1

                # normalize: outT[:, h, l] = out_ps[:, l] / sum_ps[l]
                recip = attn_sm.tile([1, L], F32R, tag="recip", name=f"rc{b}{h}")
                nc.vector.reciprocal(recip[:], sum_ps)
                bc_t = bc_psum.tile([128, L], F32, tag="bc", name=f"bc{b}{h}")
                bc_ps = bc_t[:]
                nc.tensor.matmul(bc_ps, ones_row32[:], recip[:], start=True, stop=True)
                ocopy = attn_sm.tile([128, L], F32, tag="ocopy", name=f"oc{b}{h}")
                nc.vector.tensor_copy(ocopy[:], out_ps)
                nc.vector.tensor_mul(outT_b[:, h, :], ocopy[:], bc_ps)

            # Output projection for this batch (staged into 4-block groups)
            yT_g = _grouped_rows(t["yT"])
            for g in range(KT // GRP):
                y_sb = ysb_pool.tile([128, GRP, L], F32, tag="ysb", name=f"y{b}_{g}")
                for j in range(GRP):
                    m = g * GRP + j
                    y_ps = wo_psum.tile([128, L], F32, tag="y", name=f"yp{b}_{m}")
                    for h in range(HPC):
                        nc.tensor.matmul(
                            y_ps[:],
                            wo_sb[:, h, m * 128 : (m + 1) * 128],
                            outT_b[:, h, :],
                            start=(h == 0),
                            stop=(h == HPC - 1),
                        )
                    nc.vector.tensor_copy(y_sb[:, j, :], y_ps[:])
                nc.sync.dma_start(yT_g[g][:, :, b * L : (b + 1) * L], y_sb[:])


def build_program(reps_loop=False):
    nc = bacc.Bacc(
        "TRN2",
        target_bir_lowering=False,
        debug=False,
        enable_asserts=False,
        num_devices=N_CORES,
    )
    t = _declare_io(nc, with_reps=reps_loop)

    with (
        nc.allow_low_precision(reason="deliberate fp16/fp32r compute"),
        tile.TileContext(nc) as tc,
    ):
        with contextlib.ExitStack() as ctx:
            consts_pool = ctx.enter_context(tc.tile_pool(name="consts", bufs=1))
            xt_pool = ctx.enter_context(tc.tile_pool(name="xt", bufs=1))
            wo_pool = ctx.enter_context(tc.tile_pool(name="wop", bufs=1))
            qkv_pool = ctx.enter_context(tc.tile_pool(name="qkv", bufs=2))

            qw_ld = consts_pool.tile([1, Dh], F32)
            nc.sync.dma_start(qw_ld[:], t["qw"][:])
            qw_sb = consts_pool.tile([1, Dh], F32R)
            nc.vector.tensor_copy(qw_sb[:], qw_ld[:])
            kw_ld = consts_pool.tile([1, Dh], F32)
            nc.sync.dma_start(kw_ld[:], t["kw"][:])
            kw_sb = consts_pool.tile([1, Dh], F32R)
            nc.vector.tensor_copy(kw_sb[:], kw_ld[:])
            mask_sb = consts_pool.tile([128, 2, L], DT)
            nc.sync.dma_start(mask_sb[:], t["maskt"][:])
            ones_col16 = consts_pool.tile([128, 1], DT)
            nc.vector.memset(ones_col16[:], 1.0)
            ones_stage = consts_pool.tile([128, 128], F32)
            nc.vector.memset(ones_stage[:], 1.0)
            ones_col32 = consts_pool.tile([128, 1], F32R)
            nc.vector.tensor_copy(ones_col32[:], ones_stage[:, 0:1])
            ones_row32 = consts_pool.tile([1, 128], F32R)
            nc.vector.tensor_copy(ones_row32[:], ones_stage[0:1, :])
            eps_sb = consts_pool.tile([1, 1], F32)
            nc.vector.memset(eps_sb[:], EPS)

            consts = (qw_sb, kw_sb, mask_sb, ones_col16, ones_col32, ones_row32, eps_sb)
            pools = {"qkv": qkv_pool, "xt": xt_pool, "wo": wo_pool}

            if reps_loop:
                reps_sb = consts_pool.tile([1, 1], mybir.dt.uint32)
                nc.sync.dma_start(reps_sb[:], t["reps"][:])
                reps_regs = nc.alloc_registers("reps_regs")
                nc.regs_load(reps_regs, reps_sb[0:1, 0:1])
                reps_val = nc.snap(reps_regs, donate=True, min_val=1, max_val=1 << 20)
                with tc.For_i(0, reps_val, 1):
                    _emit_body(nc, tc, t, consts, pools)
            else:
                _emit_body(nc, tc, t, consts, pools)

    nc.compile()
    return nc


def prep_inputs(x, cache_k, cache_v, Wq, Wk, Wv, Wo, q_norm_w, k_norm_w, sink):
    """Host-side sharding/layout prep. Returns in_maps for the 8 cores."""
    x = np.asarray(x, dtype=np.float32)
    cache_k = np.asarray(cache_k, dtype=np.float32)
    cache_v = np.asarray(cache_v, dtype=np.float32)
    Wq = np.asarray(Wq, dtype=np.float32)
    Wk = np.asarray(Wk, dtype=np.float32)
    Wv = np.asarray(Wv, dtype=np.float32)
    Wo = np.asarray(Wo, dtype=np.float32)
    q_norm_w = np.asarray(q_norm_w, dtype=np.float32)
    k_norm_w = np.asarray(k_norm_w, dtype=np.float32)
    sink = int(sink)
    assert sink == SINK, f"kernel hardcodes sink={SINK}, got {sink}"

    xT = np.ascontiguousarray(x.reshape(TOK, D).T.astype(DT_NP))  # [D, TOK]

    # kept old-cache rows: [0:sink] ++ [sink+L : S]
    ck = np.concatenate([cache_k[:, :SINK], cache_k[:, SINK + L :]], axis=1)
    cv = np.concatenate([cache_v[:, :SINK], cache_v[:, SINK + L :]], axis=1)

    qw = np.ascontiguousarray(q_norm_w.reshape(1, Dh))
    kw = np.ascontiguousarray(k_norm_w.reshape(1, Dh))

    # mask[p, c, l] = 1 if new-chunk position c*128+p is visible to query l
    pi = np.arange(128)[:, None, None]
    cc = np.arange(2)[None, :, None]
    ll = np.arange(L)[None, None, :]
    maskt = ((cc * 128 + pi) <= ll).astype(DT_NP)

    in_maps = []
    for core in range(N_CORES):
        hs = core * HPC
        fs = hs * Dh  # feature start column
        fe = fs + HPC * Dh
        wq_c = np.ascontiguousarray(Wq[:, fs:fe].astype(DT_NP))
        wk_c = np.ascontiguousarray(Wk[:, fs:fe].astype(DT_NP))
        wv_c = np.ascontiguousarray(Wv[:, fs:fe].astype(DT_NP))
        wo_c = np.ascontiguousarray(
            Wo[fs:fe, :].reshape(HPC, 128, D).transpose(1, 0, 2).astype(DT_NP)
        )  # [128, HPC, D]
        kTc = np.ascontiguousarray(
            ck[:, :, hs : hs + HPC, :].transpose(0, 2, 3, 1).astype(DT_NP)
        )  # [B, HPC, Dh, KEEP]
        vc = np.ascontiguousarray(
            cv[:, :, hs : hs + HPC, :].reshape(B, KEEP, HPC * Dh).astype(DT_NP)
        )  # [B, KEEP, HPC*Dh]
        in_maps.append(
            {
                "xT": xT,
                "wq": wq_c,
                "wk": wk_c,
                "wv": wv_c,
                "wo": wo_c,
                "kTc": kTc,
                "vc": vc,
                "qw": qw,
                "kw": kw,
                "maskt": maskt,
            }
        )
    return in_maps


_PROGRAM_CACHE = {}


def _get_program(reps_loop=False):
    key = bool(reps_loop)
    if key not in _PROGRAM_CACHE:
        _PROGRAM_CACHE[key] = build_program(reps_loop=key)
    return _PROGRAM_CACHE[key]


def kernel(**inputs) -> np.ndarray:
    in_maps = prep_inputs(**inputs)
    nc = _get_program(reps_loop=False)
    res = run_bass_kernel_spmd(nc, in_maps, core_ids=list(range(N_CORES)))
    yT = np.zeros((D, TOK), np.float64)
    for c in range(N_CORES):
        yT += res.results[c]["yT"].astype(np.float64)
    y = yT.T.reshape(B, L, D).astype(np.float32)
    return y


if __name__ == "__main__":
    rng = np.random.default_rng(0)
    inputs = {
        "x": rng.standard_normal((B, L, D), dtype=np.float32),
        "cache_k": rng.standard_normal((B, S, H, Dh), dtype=np.float32),
        "cache_v": rng.standard_normal((B, S, H, Dh), dtype=np.float32),
        "Wq": (rng.standard_normal((D, D), dtype=np.float32) * 0.02),
        "Wk": (rng.standard_normal((D, D), dtype=np.float32) * 0.02),
        "Wv": (rng.standard_normal((D, D), dtype=np.float32) * 0.02),
        "Wo": (rng.standard_normal((D, D), dtype=np.float32) * 0.02),
        "q_norm_w": np.ones(Dh, np.float32),
        "k_norm_w": np.ones(Dh, np.float32),
        "sink": SINK,
    }
    y = kernel(**inputs)
    print("y", y.shape, y.dtype, float(np.abs(y).mean()))
